# revision 8
# baseline (speedup 1.0000x reference)
"""Trainium2 Bass kernel for nn_BCE_Loss (focal-style BCE-with-logits, mean).

Reference math per anchor row x[0:3] (logits) and integer target c:
    col = 0 if c==1 else 1 if c==3 else 2
    t   = one_hot(col, 3)
    loss_el = (x - t)^2 * softplus(x * (1 - 2t))
    out = mean(loss_el)

Host-side plane reorder removes the one-hot entirely: per anchor ship
    z0 = -x[col]   (the selected logit, negated)
    z1, z2 = the two non-selected logits
Then elementwise loss = (z + 1_{plane0})^2 * softplus(z), so
    S = sum_all z^2*sp + 2*sum_pl0 z*sp + sum_pl0 sp,   sp = softplus(z)

Device per tile (class-planar [P, 3s] slabs, bf16):
    DMA  z tile via the sync-engine HWDGE ring (no SWDGE - DVE 2x ops
         starve the gpsimd descriptor generator)
    ACT  E = Exp(z) -> PSUM f32 (this toolchain has no Softplus table;
         sp needs Exp+Ln), then sp = Ln(E+1): plane-0 pass carries
         accum_out (per-partition sum of sp plane0); planes 1-2 in one
         second Ln pass.  ACT runs 2 passes over every element - the
         engine roofline of this kernel (~41us/core busy).
    DVE  Q = z * sp, single tensor_tensor in 2x_1P bf16 mode
    PE   per 128-col chunk: diag-accumulate sum Q*z into psA [128,128];
         plane-0 chunks add an N=1 ones-matmul into psA0 [128,1] for
         sum_pl0 Q
Epilogue: S = sum(diag(psA)) + 2*sum(psA0) + sum(accsp0), partition-
reduced by a ones matmul; host sums the 8 per-core partials / count.

Sharding: pure data-parallel, contiguous anchor blocks per core.
"""

import numpy as np

import concourse.bacc as bacc
import concourse.bass as bass
import concourse.mybir as mybir
from concourse import bass_utils
from concourse.alu_op_type import AluOpType
from concourse.tile import TileContext

N_CORES = 8
N_ANCHORS = 8388608
N_CLASSES = 3
N_SHARD = N_ANCHORS // N_CORES  # 1048576
P = 128  # SBUF partitions
A_PART = N_SHARD // P  # 8192 anchors per partition
SIZES = [256, 384] + [512] * 14 + [384]
assert sum(SIZES) == A_PART
NT = len(SIZES)
MM = 128  # diag-trick matmul chunk width


class _Bacc(bacc.Bacc):
    """Bacc with the ACT table pinned to natural_log_exp_and_others.

    The default chooser puts Exp in exp_and_others and Ln in natural_log,
    reloading tables every tile (~2.7us each). Both live in
    natural_log_exp_and_others; emptying every other set (positions kept -
    act_func_set_id is the index into act_info.json) forces one load."""

    _ACT_SET = "natural_log_exp_and_others"

    def insert_act_table_loads(self):
        import bass_rust as _bass_rust

        from concourse.hw_specs import get_activation_tables

        has_activation = any(
            isinstance(i, mybir.InstActivation)
            for b in self.main_func.blocks
            for i in b.instructions
        )
        if not has_activation:
            return
        tables = [
            (name, (fns if name == self._ACT_SET else set()))
            for name, fns in get_activation_tables(self.m.arch).items()
        ]
        _bass_rust.insert_act_table_loads(self, tables)


def _build_nc() -> bass.Bass:
    nc = _Bacc("TRN2", target_bir_lowering=False, num_swdge_queues=4)
    z = nc.dram_tensor(
        "z", [N_CLASSES * N_SHARD], mybir.dt.bfloat16, kind="ExternalInput"
    )
    msk = nc.dram_tensor("msk", [P, MM], mybir.dt.bfloat16, kind="ExternalInput")
    out = nc.dram_tensor("out", [1], mybir.dt.float32, kind="ExternalOutput")

    # class-planar: element (j, p, a) -> partition p, plane j, col a
    zv = z.rearrange("(j p a) -> p j a", j=N_CLASSES, p=P)

    with TileContext(nc) as tc:
        with (
            tc.tile_pool(name="io", bufs=3) as io,
            tc.tile_pool(name="spp", bufs=3) as spp,
            tc.tile_pool(name="qp", bufs=3) as qp,
            tc.tile_pool(name="singles", bufs=1) as singles,
            tc.tile_pool(name="psum", bufs=1, space="PSUM") as psum,
            tc.tile_pool(name="psE", bufs=2, space="PSUM") as psE,
        ):
            ones_f = singles.tile([P, 1], mybir.dt.float32)
            nc.vector.memset(ones_f, 1.0)
            ones_b = singles.tile([P, 1], mybir.dt.bfloat16)
            nc.vector.memset(ones_b, 1.0)
            msk_t = singles.tile([P, MM], mybir.dt.bfloat16)
            nc.scalar.dma_start(out=msk_t, in_=msk[:, :])
            accsp0 = singles.tile([P, NT], mybir.dt.float32)

            # one PSUM bank: cols 0..127 diag accum, col 128 plane0 sum(Q)
            psAB = psum.tile([P, MM + 1], mybir.dt.float32)
            psA = psAB[:, 0:MM]
            psA0 = psAB[:, MM : MM + 1]

            n_chunks = sum(N_CLASSES * s // MM for s in SIZES)
            n_p0 = sum(s // MM for s in SIZES)
            chunk_id = 0
            p0_id = 0
            off = 0
            for i, size in enumerate(SIZES):
                F = N_CLASSES * size
                zt = io.tile([P, F], mybir.dt.bfloat16)
                z3 = zt.rearrange("p (j t) -> p j t", j=N_CLASSES)
                nc.sync.dma_start(out=z3, in_=zv[:, :, off : off + size])

                # softplus = Ln(Exp(z) + 1); E lives in PSUM f32
                E = psE.tile([P, F], mybir.dt.float32)
                nc.scalar.activation(
                    out=E, in_=zt, func=mybir.ActivationFunctionType.Exp)
                sp = spp.tile([P, F], mybir.dt.bfloat16)
                # plane 0 with per-partition accumulate of sp
                nc.scalar.activation(
                    out=sp[:, 0:size], in_=E[:, 0:size],
                    func=mybir.ActivationFunctionType.Ln, bias=1.0,
                    accum_out=accsp0[:, i : i + 1])
                # planes 1..2 in one pass
                nc.scalar.activation(
                    out=sp[:, size:F], in_=E[:, size:F],
                    func=mybir.ActivationFunctionType.Ln, bias=1.0)

                q = qp.tile([P, F], mybir.dt.bfloat16)
                nc.vector.tensor_tensor(out=q, in0=zt, in1=sp, op=AluOpType.mult)

                for c in range(F // MM):
                    sl = slice(c * MM, (c + 1) * MM)
                    nc.tensor.matmul(
                        psA[:, :], q[:, sl], zt[:, sl],
                        start=(chunk_id == 0),
                        stop=(chunk_id == n_chunks - 1))
                    chunk_id += 1
                    if c < size // MM:  # plane-0 chunk: accumulate sum(Q)
                        nc.tensor.matmul(
                            psA0[:, :], q[:, sl], ones_b[:, :],
                            start=(p0_id == 0),
                            stop=(p0_id == n_p0 - 1))
                        p0_id += 1
                off += size

            # epilogue: S = sum diag(psA) + 2*sum(psA0) + sum(accsp0)
            dm = singles.tile([P, MM], mybir.dt.float32)
            nc.vector.tensor_tensor(out=dm, in0=psA, in1=msk_t, op=AluOpType.mult)
            r1 = singles.tile([P, 1], mybir.dt.float32)
            nc.vector.tensor_reduce(
                out=r1, in_=dm, axis=mybir.AxisListType.X, op=AluOpType.add)
            racc = singles.tile([P, 1], mybir.dt.float32)
            nc.vector.tensor_reduce(
                out=racc, in_=accsp0, axis=mybir.AxisListType.X, op=AluOpType.add)
            t1 = singles.tile([P, 1], mybir.dt.float32)
            nc.vector.scalar_tensor_tensor(
                out=t1, in0=psA0, scalar=2.0, in1=r1,
                op0=AluOpType.mult, op1=AluOpType.add)
            tot = singles.tile([P, 1], mybir.dt.float32)
            nc.vector.tensor_tensor(out=tot, in0=t1, in1=racc, op=AluOpType.add)

            psT = psum.tile([1, 1], mybir.dt.float32)
            nc.tensor.matmul(psT[:, :], ones_f[:, :], tot[:, :], start=True, stop=True)
            res = singles.tile([1, 1], mybir.dt.float32)
            nc.vector.tensor_copy(out=res, in_=psT)
            nc.sync.dma_start(out=out[:], in_=res[0, :])

    nc.compile()
    return nc


_cache: dict[str, bass.Bass] = {}
last_results = None  # BassKernelResults of the most recent run (for test.py)


def _get_nc() -> bass.Bass:
    if "nc" not in _cache:
        _cache["nc"] = _build_nc()
    return _cache["nc"]


def _msk_bf16() -> np.ndarray:
    import ml_dtypes

    m = np.zeros((P, MM), dtype=np.float32)
    idx = np.arange(P)
    m[idx, idx] = 1.0
    return m.astype(ml_dtypes.bfloat16)


def _host_planes(pred: np.ndarray, targ: np.ndarray) -> np.ndarray:
    """[3, N] f32: plane0 = -selected logit, planes 1/2 = the others."""
    col = np.where(targ == 1, 0, np.where(targ == 3, 1, 2)).astype(np.int64)
    sel = pred[np.arange(pred.shape[0]), col]
    m0 = col == 0
    m2 = col == 2
    z = np.empty((N_CLASSES, pred.shape[0]), dtype=np.float32)
    z[0] = -sel
    z[1] = np.where(m0, pred[:, 1], pred[:, 0])
    z[2] = np.where(m2, pred[:, 1], pred[:, 2])
    return z


def kernel(pred: np.ndarray, targ: np.ndarray, *, trace: bool = False) -> np.ndarray:
    global last_results
    import ml_dtypes

    pred = np.ascontiguousarray(np.asarray(pred, dtype=np.float32))
    targ = np.asarray(targ)
    assert pred.shape == (N_ANCHORS, N_CLASSES), pred.shape
    assert targ.shape == (N_ANCHORS,), targ.shape

    zf = _host_planes(pred, targ)
    zb = zf.astype(ml_dtypes.bfloat16)

    nc = _get_nc()
    msk = _msk_bf16()

    in_maps = []
    for c in range(N_CORES):
        sl = slice(c * N_SHARD, (c + 1) * N_SHARD)
        # per-core class-planar block, flat (j p a) order
        zc = np.ascontiguousarray(zb[:, sl]).reshape(-1)
        in_maps.append({"z": zc, "msk": msk})

    res = bass_utils.run_bass_kernel_spmd(
        nc, in_maps, core_ids=list(range(N_CORES)), trace=trace
    )
    last_results = res

    total = np.float64(0.0)
    for r in res.results:
        total += np.float64(r["out"][0])
    mean = total / (N_ANCHORS * N_CLASSES)
    return np.float32(mean)


# revision 9
# speedup vs baseline: 1.3583x; 1.3583x over previous
"""Trainium2 Bass kernel for nn_BCE_Loss (focal-style BCE-with-logits, mean).

Reference math per anchor row x[0:3] (logits) and integer target c:
    col = 0 if c==1 else 1 if c==3 else 2
    t   = one_hot(col, 3)
    loss_el = (x - t)^2 * softplus(x * (1 - 2t))
    out = mean(loss_el)

Host-side plane reorder removes the one-hot entirely: per anchor ship
    z0 = -x[col]   (the selected logit, negated)
    z1, z2 = the two non-selected logits
Then elementwise loss = (z + 1_{plane0})^2 * softplus(z), so
    S = sum_all z^2*sp + 2*sum_pl0 z*sp + sum_pl0 sp,   sp = softplus(z)

Device per tile (class-planar [P, 3s] slabs, bf16):
    DMA  z tile via the sync-engine HWDGE ring (no SWDGE - DVE 2x ops
         starve the gpsimd descriptor generator)
    ACT  E = Exp(z) -> SBUF f32 (no Softplus table on this toolchain, so
         softplus = Exp+Ln: 2 passes over every element - the engine
         roofline of this kernel; SBUF E keeps tiles large, 2 ACT
         instructions per tile amortizes the ~312ns/instr ACT overhead)
         then sp = Ln(E + 1) -> SBUF bf16
    DVE  Q = z * sp (tensor_tensor, 2x_1P bf16); grouped 16-way reduces
         of sp plane0 and Q plane0 (bf16 partials, 2x) for the delta
         terms
    PE   per 128-col chunk: diag-accumulate sum Q*z into psA [128,128]
Epilogue: S = sum(diag(psA)) + 2*sum(qred) + sum(spred), partition-
reduced by a ones matmul; host sums the 8 per-core partials / count.

Sharding: pure data-parallel, contiguous anchor blocks per core.
"""

import numpy as np

import concourse.bacc as bacc
import concourse.bass as bass
import concourse.mybir as mybir
from concourse import bass_utils
from concourse.alu_op_type import AluOpType
from concourse.tile import TileContext

N_CORES = 8
N_ANCHORS = 8388608
N_CLASSES = 3
N_SHARD = N_ANCHORS // N_CORES  # 1048576
P = 128  # SBUF partitions
A_PART = N_SHARD // P  # 8192 anchors per partition
SIZES = [1024, 1792, 1792, 1792, 1792]
assert sum(SIZES) == A_PART
NT = len(SIZES)
MM = 128  # diag-trick matmul chunk width
G = 16  # plane-0 grouped-reduce partial count per tile


class _Bacc(bacc.Bacc):
    """Bacc with the ACT table pinned to natural_log_exp_and_others.

    The default chooser puts Exp in exp_and_others and Ln in natural_log,
    reloading tables every tile (~2.7us each). Both live in
    natural_log_exp_and_others; emptying every other set (positions kept -
    act_func_set_id is the index into act_info.json) forces one load."""

    _ACT_SET = "natural_log_exp_and_others"

    def insert_act_table_loads(self):
        import bass_rust as _bass_rust

        from concourse.hw_specs import get_activation_tables

        has_activation = any(
            isinstance(i, mybir.InstActivation)
            for b in self.main_func.blocks
            for i in b.instructions
        )
        if not has_activation:
            return
        tables = [
            (name, (fns if name == self._ACT_SET else set()))
            for name, fns in get_activation_tables(self.m.arch).items()
        ]
        _bass_rust.insert_act_table_loads(self, tables)


def _build_nc() -> bass.Bass:
    nc = _Bacc("TRN2", target_bir_lowering=False, num_swdge_queues=4)
    z = nc.dram_tensor(
        "z", [N_CLASSES * N_SHARD], mybir.dt.bfloat16, kind="ExternalInput"
    )
    msk = nc.dram_tensor("msk", [P, MM], mybir.dt.bfloat16, kind="ExternalInput")
    out = nc.dram_tensor("out", [1], mybir.dt.float32, kind="ExternalOutput")

    # class-planar: element (j, p, a) -> partition p, plane j, col a
    zv = z.rearrange("(j p a) -> p j a", j=N_CLASSES, p=P)

    with TileContext(nc) as tc:
        with (
            tc.tile_pool(name="io", bufs=3) as io,
            tc.tile_pool(name="ep", bufs=3) as ep,
            tc.tile_pool(name="spp", bufs=3) as spp,
            tc.tile_pool(name="qp", bufs=3) as qp,
            tc.tile_pool(name="singles", bufs=1) as singles,
            tc.tile_pool(name="psum", bufs=1, space="PSUM") as psum,
        ):
            ones_f = singles.tile([P, 1], mybir.dt.float32)
            nc.vector.memset(ones_f, 1.0)
            msk_t = singles.tile([P, MM], mybir.dt.bfloat16)
            nc.scalar.dma_start(out=msk_t, in_=msk[:, :])
            spredt = singles.tile([P, G * NT], mybir.dt.bfloat16)
            qredt = singles.tile([P, G * NT], mybir.dt.bfloat16)

            psA = psum.tile([P, MM], mybir.dt.float32)

            n_chunks = sum(N_CLASSES * s // MM for s in SIZES)
            chunk_id = 0
            off = 0
            for i, size in enumerate(SIZES):
                F = N_CLASSES * size
                zt = io.tile([P, F], mybir.dt.bfloat16)
                z3 = zt.rearrange("p (j t) -> p j t", j=N_CLASSES)
                nc.sync.dma_start(out=z3, in_=zv[:, :, off : off + size])

                # softplus = Ln(Exp(z) + 1)
                E = ep.tile([P, F], mybir.dt.float32)
                nc.scalar.activation(
                    out=E, in_=zt, func=mybir.ActivationFunctionType.Exp)
                sp = spp.tile([P, F], mybir.dt.bfloat16)
                nc.scalar.activation(
                    out=sp, in_=E, func=mybir.ActivationFunctionType.Ln,
                    bias=1.0)

                q = qp.tile([P, F], mybir.dt.bfloat16)
                nc.vector.tensor_tensor(out=q, in0=zt, in1=sp, op=AluOpType.mult)

                # plane-0 delta terms: grouped partial sums (bf16, 2x mode)
                with nc.allow_low_precision("grouped bf16 partials, ~128 terms"):
                    nc.vector.tensor_reduce(
                        out=spredt[:, i * G : (i + 1) * G],
                        in_=sp[:, 0:size].rearrange("p (g w) -> p g w", g=G),
                        axis=mybir.AxisListType.X, op=AluOpType.add)
                    nc.vector.tensor_reduce(
                        out=qredt[:, i * G : (i + 1) * G],
                        in_=q[:, 0:size].rearrange("p (g w) -> p g w", g=G),
                        axis=mybir.AxisListType.X, op=AluOpType.add)

                for c in range(F // MM):
                    sl = slice(c * MM, (c + 1) * MM)
                    nc.tensor.matmul(
                        psA[:, :], q[:, sl], zt[:, sl],
                        start=(chunk_id == 0),
                        stop=(chunk_id == n_chunks - 1))
                    chunk_id += 1
                off += size

            # epilogue: S = sum diag(psA) + 2*sum(qredt) + sum(spredt)
            dm = singles.tile([P, MM], mybir.dt.float32)
            nc.vector.tensor_tensor(out=dm, in0=psA, in1=msk_t, op=AluOpType.mult)
            r1 = singles.tile([P, 1], mybir.dt.float32)
            nc.vector.tensor_reduce(
                out=r1, in_=dm, axis=mybir.AxisListType.X, op=AluOpType.add)
            rsp = singles.tile([P, 1], mybir.dt.float32)
            nc.vector.tensor_reduce(
                out=rsp, in_=spredt, axis=mybir.AxisListType.X, op=AluOpType.add)
            rq = singles.tile([P, 1], mybir.dt.float32)
            nc.vector.tensor_reduce(
                out=rq, in_=qredt, axis=mybir.AxisListType.X, op=AluOpType.add)
            t1 = singles.tile([P, 1], mybir.dt.float32)
            nc.vector.scalar_tensor_tensor(
                out=t1, in0=rq, scalar=2.0, in1=r1,
                op0=AluOpType.mult, op1=AluOpType.add)
            tot = singles.tile([P, 1], mybir.dt.float32)
            nc.vector.tensor_tensor(out=tot, in0=t1, in1=rsp, op=AluOpType.add)

            psT = psum.tile([1, 1], mybir.dt.float32)
            nc.tensor.matmul(psT[:, :], ones_f[:, :], tot[:, :], start=True, stop=True)
            res = singles.tile([1, 1], mybir.dt.float32)
            nc.vector.tensor_copy(out=res, in_=psT)
            nc.sync.dma_start(out=out[:], in_=res[0, :])

    nc.compile()
    return nc


_cache: dict[str, bass.Bass] = {}
last_results = None  # BassKernelResults of the most recent run (for test.py)


def _get_nc() -> bass.Bass:
    if "nc" not in _cache:
        _cache["nc"] = _build_nc()
    return _cache["nc"]


def _msk_bf16() -> np.ndarray:
    import ml_dtypes

    m = np.zeros((P, MM), dtype=np.float32)
    idx = np.arange(P)
    m[idx, idx] = 1.0
    return m.astype(ml_dtypes.bfloat16)


def _host_planes(pred: np.ndarray, targ: np.ndarray) -> np.ndarray:
    """[3, N] f32: plane0 = -selected logit, planes 1/2 = the others."""
    col = np.where(targ == 1, 0, np.where(targ == 3, 1, 2)).astype(np.int64)
    sel = pred[np.arange(pred.shape[0]), col]
    m0 = col == 0
    m2 = col == 2
    z = np.empty((N_CLASSES, pred.shape[0]), dtype=np.float32)
    z[0] = -sel
    z[1] = np.where(m0, pred[:, 1], pred[:, 0])
    z[2] = np.where(m2, pred[:, 1], pred[:, 2])
    return z


def kernel(pred: np.ndarray, targ: np.ndarray, *, trace: bool = False) -> np.ndarray:
    global last_results
    import ml_dtypes

    pred = np.ascontiguousarray(np.asarray(pred, dtype=np.float32))
    targ = np.asarray(targ)
    assert pred.shape == (N_ANCHORS, N_CLASSES), pred.shape
    assert targ.shape == (N_ANCHORS,), targ.shape

    zf = _host_planes(pred, targ)
    zb = zf.astype(ml_dtypes.bfloat16)

    nc = _get_nc()
    msk = _msk_bf16()

    in_maps = []
    for c in range(N_CORES):
        sl = slice(c * N_SHARD, (c + 1) * N_SHARD)
        # per-core class-planar block, flat (j p a) order
        zc = np.ascontiguousarray(zb[:, sl]).reshape(-1)
        in_maps.append({"z": zc, "msk": msk})

    res = bass_utils.run_bass_kernel_spmd(
        nc, in_maps, core_ids=list(range(N_CORES)), trace=trace
    )
    last_results = res

    total = np.float64(0.0)
    for r in res.results:
        total += np.float64(r["out"][0])
    mean = total / (N_ANCHORS * N_CLASSES)
    return np.float32(mean)


# revision 11
# speedup vs baseline: 1.4291x; 1.0521x over previous
"""Trainium2 Bass kernel for nn_BCE_Loss (focal-style BCE-with-logits, mean).

Reference math per anchor row x[0:3] (logits) and integer target c:
    col = 0 if c==1 else 1 if c==3 else 2
    t   = one_hot(col, 3)
    loss_el = (x - t)^2 * softplus(x * (1 - 2t))
    out = mean(loss_el)

Host-side plane reorder removes the one-hot entirely: per anchor ship
    z0 = -x[col]   (the selected logit, negated)
    z1, z2 = the two non-selected logits
Then elementwise loss = (z + 1_{plane0})^2 * softplus(z), so
    S = sum_all z^2*sp + 2*sum_pl0 z*sp + sum_pl0 sp,   sp = softplus(z)

Device per tile (class-planar [P, 3s] slabs, bf16):
    DMA  z tile via the sync-engine HWDGE ring (no SWDGE - DVE 2x ops
         starve the gpsimd descriptor generator)
    ACT  E = Exp(z) -> SBUF f32 (no Softplus table on this toolchain, so
         softplus = Exp+Ln: 2 passes over every element - the engine
         roofline of this kernel; SBUF E keeps tiles large, 2 ACT
         instructions per tile amortizes the ~312ns/instr ACT overhead)
         then sp = Ln(E + 1) -> SBUF bf16
    DVE  Q = z * sp (tensor_tensor, 2x_1P bf16); grouped 16-way reduces
         of sp plane0 and Q plane0 (bf16 partials, 2x) for the delta
         terms
    PE   per 128-col chunk: diag-accumulate sum Q*z into psA [128,128]
Epilogue: S = sum(diag(psA)) + 2*sum(qred) + sum(spred), partition-
reduced by a ones matmul; host sums the 8 per-core partials / count.

Sharding: pure data-parallel, contiguous anchor blocks per core.
"""

import numpy as np

import concourse.bacc as bacc
import concourse.bass as bass
import concourse.mybir as mybir
from concourse import bass_utils
from concourse.alu_op_type import AluOpType
from concourse.tile import TileContext

N_CORES = 8
N_ANCHORS = 8388608
N_CLASSES = 3
N_SHARD = N_ANCHORS // N_CORES  # 1048576
P = 128  # SBUF partitions
A_PART = N_SHARD // P  # 8192 anchors per partition
# big tiles early (amortize ~312ns/instr ACT overhead), small tiles at the
# end (the post-ACT tail is DVE-Q + cold-PE matmuls of the LAST tile only)
SIZES = [768, 1792, 1792, 1792, 1408, 384, 256]
assert sum(SIZES) == A_PART
NT = len(SIZES)
MM = 128  # diag-trick matmul chunk width
G = 16  # plane-0 grouped-reduce partial count per tile


class _Bacc(bacc.Bacc):
    """Bacc with the ACT table pinned to natural_log_exp_and_others.

    The default chooser puts Exp in exp_and_others and Ln in natural_log,
    reloading tables every tile (~2.7us each). Both live in
    natural_log_exp_and_others; emptying every other set (positions kept -
    act_func_set_id is the index into act_info.json) forces one load."""

    _ACT_SET = "natural_log_exp_and_others"

    def insert_act_table_loads(self):
        import bass_rust as _bass_rust

        from concourse.hw_specs import get_activation_tables

        has_activation = any(
            isinstance(i, mybir.InstActivation)
            for b in self.main_func.blocks
            for i in b.instructions
        )
        if not has_activation:
            return
        tables = [
            (name, (fns if name == self._ACT_SET else set()))
            for name, fns in get_activation_tables(self.m.arch).items()
        ]
        _bass_rust.insert_act_table_loads(self, tables)


def _build_nc() -> bass.Bass:
    nc = _Bacc("TRN2", target_bir_lowering=False, num_swdge_queues=4)
    z = nc.dram_tensor(
        "z", [N_CLASSES * N_SHARD], mybir.dt.bfloat16, kind="ExternalInput"
    )
    msk = nc.dram_tensor("msk", [P, MM], mybir.dt.bfloat16, kind="ExternalInput")
    out = nc.dram_tensor("out", [1], mybir.dt.float32, kind="ExternalOutput")

    # class-planar: element (j, p, a) -> partition p, plane j, col a
    zv = z.rearrange("(j p a) -> p j a", j=N_CLASSES, p=P)

    with TileContext(nc) as tc:
        with (
            tc.tile_pool(name="io", bufs=3) as io,
            tc.tile_pool(name="ep", bufs=3) as ep,
            tc.tile_pool(name="spp", bufs=3) as spp,
            tc.tile_pool(name="qp", bufs=3) as qp,
            tc.tile_pool(name="singles", bufs=1) as singles,
            tc.tile_pool(name="psum", bufs=1, space="PSUM") as psum,
        ):
            ones_f = singles.tile([P, 1], mybir.dt.float32)
            nc.vector.memset(ones_f, 1.0)
            msk_t = singles.tile([P, MM], mybir.dt.bfloat16)
            nc.scalar.dma_start(out=msk_t, in_=msk[:, :])
            spredt = singles.tile([P, G * NT], mybir.dt.bfloat16)
            qredt = singles.tile([P, G * NT], mybir.dt.bfloat16)

            psA = psum.tile([P, MM], mybir.dt.float32)

            n_chunks = sum(N_CLASSES * s // MM for s in SIZES)
            chunk_id = 0
            off = 0
            for i, size in enumerate(SIZES):
                F = N_CLASSES * size
                zt = io.tile([P, F], mybir.dt.bfloat16)
                z3 = zt.rearrange("p (j t) -> p j t", j=N_CLASSES)
                nc.sync.dma_start(out=z3, in_=zv[:, :, off : off + size])

                # softplus = Ln(Exp(z) + 1)
                E = ep.tile([P, F], mybir.dt.float32)
                nc.scalar.activation(
                    out=E, in_=zt, func=mybir.ActivationFunctionType.Exp)
                sp = spp.tile([P, F], mybir.dt.bfloat16)
                nc.scalar.activation(
                    out=sp, in_=E, func=mybir.ActivationFunctionType.Ln,
                    bias=1.0)

                # Q in halves so PE can start on the first half early
                q = qp.tile([P, F], mybir.dt.bfloat16)
                H = F // 2 // MM * MM
                nc.vector.tensor_tensor(
                    out=q[:, 0:H], in0=zt[:, 0:H], in1=sp[:, 0:H],
                    op=AluOpType.mult)
                nc.vector.tensor_tensor(
                    out=q[:, H:F], in0=zt[:, H:F], in1=sp[:, H:F],
                    op=AluOpType.mult)

                # plane-0 delta terms: grouped partial sums (bf16, 2x mode)
                with nc.allow_low_precision("grouped bf16 partials, ~128 terms"):
                    nc.vector.tensor_reduce(
                        out=spredt[:, i * G : (i + 1) * G],
                        in_=sp[:, 0:size].rearrange("p (g w) -> p g w", g=G),
                        axis=mybir.AxisListType.X, op=AluOpType.add)
                    nc.vector.tensor_reduce(
                        out=qredt[:, i * G : (i + 1) * G],
                        in_=q[:, 0:size].rearrange("p (g w) -> p g w", g=G),
                        axis=mybir.AxisListType.X, op=AluOpType.add)

                for c in range(F // MM):
                    sl = slice(c * MM, (c + 1) * MM)
                    nc.tensor.matmul(
                        psA[:, :], q[:, sl], zt[:, sl],
                        start=(chunk_id == 0),
                        stop=(chunk_id == n_chunks - 1))
                    chunk_id += 1
                off += size

            # epilogue: S = sum diag(psA) + 2*sum(qredt) + sum(spredt)
            dm = singles.tile([P, MM], mybir.dt.float32)
            nc.vector.tensor_tensor(out=dm, in0=psA, in1=msk_t, op=AluOpType.mult)
            r1 = singles.tile([P, 1], mybir.dt.float32)
            nc.vector.tensor_reduce(
                out=r1, in_=dm, axis=mybir.AxisListType.X, op=AluOpType.add)
            rsp = singles.tile([P, 1], mybir.dt.float32)
            nc.vector.tensor_reduce(
                out=rsp, in_=spredt, axis=mybir.AxisListType.X, op=AluOpType.add)
            rq = singles.tile([P, 1], mybir.dt.float32)
            nc.vector.tensor_reduce(
                out=rq, in_=qredt, axis=mybir.AxisListType.X, op=AluOpType.add)
            t1 = singles.tile([P, 1], mybir.dt.float32)
            nc.vector.scalar_tensor_tensor(
                out=t1, in0=rq, scalar=2.0, in1=r1,
                op0=AluOpType.mult, op1=AluOpType.add)
            tot = singles.tile([P, 1], mybir.dt.float32)
            nc.vector.tensor_tensor(out=tot, in0=t1, in1=rsp, op=AluOpType.add)

            psT = psum.tile([1, 1], mybir.dt.float32)
            nc.tensor.matmul(psT[:, :], ones_f[:, :], tot[:, :], start=True, stop=True)
            res = singles.tile([1, 1], mybir.dt.float32)
            nc.vector.tensor_copy(out=res, in_=psT)
            nc.sync.dma_start(out=out[:], in_=res[0, :])

    nc.compile()
    return nc


_cache: dict[str, bass.Bass] = {}
last_results = None  # BassKernelResults of the most recent run (for test.py)


def _get_nc() -> bass.Bass:
    if "nc" not in _cache:
        _cache["nc"] = _build_nc()
    return _cache["nc"]


def _msk_bf16() -> np.ndarray:
    import ml_dtypes

    m = np.zeros((P, MM), dtype=np.float32)
    idx = np.arange(P)
    m[idx, idx] = 1.0
    return m.astype(ml_dtypes.bfloat16)


def _host_planes(pred: np.ndarray, targ: np.ndarray) -> np.ndarray:
    """[3, N] f32: plane0 = -selected logit, planes 1/2 = the others."""
    col = np.where(targ == 1, 0, np.where(targ == 3, 1, 2)).astype(np.int64)
    sel = pred[np.arange(pred.shape[0]), col]
    m0 = col == 0
    m2 = col == 2
    z = np.empty((N_CLASSES, pred.shape[0]), dtype=np.float32)
    z[0] = -sel
    z[1] = np.where(m0, pred[:, 1], pred[:, 0])
    z[2] = np.where(m2, pred[:, 1], pred[:, 2])
    return z


def kernel(pred: np.ndarray, targ: np.ndarray, *, trace: bool = False) -> np.ndarray:
    global last_results
    import ml_dtypes

    pred = np.ascontiguousarray(np.asarray(pred, dtype=np.float32))
    targ = np.asarray(targ)
    assert pred.shape == (N_ANCHORS, N_CLASSES), pred.shape
    assert targ.shape == (N_ANCHORS,), targ.shape

    zf = _host_planes(pred, targ)
    zb = zf.astype(ml_dtypes.bfloat16)

    nc = _get_nc()
    msk = _msk_bf16()

    in_maps = []
    for c in range(N_CORES):
        sl = slice(c * N_SHARD, (c + 1) * N_SHARD)
        # per-core class-planar block, flat (j p a) order
        zc = np.ascontiguousarray(zb[:, sl]).reshape(-1)
        in_maps.append({"z": zc, "msk": msk})

    res = bass_utils.run_bass_kernel_spmd(
        nc, in_maps, core_ids=list(range(N_CORES)), trace=trace
    )
    last_results = res

    total = np.float64(0.0)
    for r in res.results:
        total += np.float64(r["out"][0])
    mean = total / (N_ANCHORS * N_CLASSES)
    return np.float32(mean)
